# revision 8
# baseline (speedup 1.0000x reference)
"""Trainium2 Bass kernel for nn_MultiHeadAttentionBlock_49967649521921.

Reference computation (per batch b, x viewed as [C=512, N=1024]):
    q = Wq @ x ; k = Wk @ x ; v = Wv @ x          (1x1 convs, biases zero)
    per head h (8 heads, hd=64):
      scores[d,e] = sum_n q_h[d,n] k_h[e,n] / 8
      attn = softmax(scores, axis=e)
      out_h[d,n]  = sum_e attn[d,e] v_h[e,n]
    y[c',s'] with c' = h*64 + n//16, s' = (n%16)*64 + d
    final = Wo @ y    -> reshape [512, 32, 32]

v4 design — Gram-matrix restructure. Because scores contract over the
SPATIAL axis, q/k projections fold into one shared Gram matrix:
    G  = X X^T                  (per batch, [512,512])
    T  = G Wk^T                 ([512,512])
    scores_hp = Wq^T-slices vs T-slices    (per head-pair [128,128],
                                            one PSUM bank per pair so the
                                            Act-exp reads never stall the
                                            next pair's matmuls)
    attn      = softmax (Act exp + accum, DVE recip, Act copy-scale)
    A'^T[c,(h,d)] = Wv^T-chunks @ blockdiag(attn^T)
    out^T[m,(h,d)] = X^T A'^T   (m-order n=16a+r -> m=64r+a makes the
                                 reference transpose(2,3) a strided copy)
    final = Wo^T-chunks^T @ y   (sh-outer so each half gates on half of y)

Schedule interleaves the two batches so softmax latency always hides
under >6us of independent matmul work:
    G0 T0 S0 | G1 T1 | X0 A0 O0 | S1 | F0 | X1 A1 O1 | F1
Loads: both HWDGE queues carry batch-0 x in >=2KB/row slices (the 1KB
descriptor path runs at ~75GB/s and starves G0); gpsimd SWDGE carries
batch-1 x.
"""

import os
import sys

import numpy as np

for _p in ("/opt/trn_rl_repo",):
    if _p not in sys.path and os.path.isdir(_p):
        sys.path.insert(0, _p)

from contextlib import ExitStack

import concourse.bass as bass
import concourse.tile as tile
from concourse import bacc
from concourse import mybir
from concourse.bass_utils import run_bass_kernel_spmd

F32 = mybir.dt.float32
BF16 = mybir.dt.bfloat16
AF = mybir.ActivationFunctionType

N_CORES = 8
B_PER_CORE = 2
C = 512
N = 1024
NH = 8
HD = 64


def _split_excess_dma_waits(nc):
    """walrus' static-DMA (PSEUDO_DMA_DIRECT2D) encoding accepts a single
    sync-wait; Bacc's generate_event_semaphores only splits waits on compute
    instructions. Move excess DMA waits onto preceding EventSemaphore
    carriers (2 waits each) on the same engine queue."""
    for f in nc.m.functions:
        for blk in f.blocks:
            changed = False
            new_insts = []
            for inst in blk.instructions:
                si = inst.sync_info
                waits = list(si.on_wait) if si is not None and si.on_wait else []
                if inst.opcode == "DMACopy" and len(waits) > 1:
                    keep, excess = waits[:1], waits[1:]
                    k = 0
                    while excess:
                        chunk, excess = excess[:2], excess[2:]
                        ev = mybir.InstEventSemaphore(
                            name=f"{inst.name}-evw{k}",
                            opcode="EventSemaphore",
                            engine=inst.engine,
                            sync_info=mybir.SyncInfo(on_wait=chunk, on_update=[]),
                        )
                        new_insts.append(ev)
                        k += 1
                    inst.sync_info = mybir.SyncInfo(
                        on_wait=keep, on_update=list(si.on_update or [])
                    )
                    changed = True
                new_insts.append(inst)
            if changed:
                blk.instructions = new_insts


def build_program():
    nc = bacc.Bacc("TRN2", target_bir_lowering=False, debug=False)

    xt_d = nc.dram_tensor("xt", [B_PER_CORE, 128, 8, C], BF16, kind="ExternalInput").ap()
    xc_d = nc.dram_tensor("xc", [B_PER_CORE, 128, 4, N], BF16, kind="ExternalInput").ap()
    wkt_d = nc.dram_tensor("wkt", [128, 4, C], BF16, kind="ExternalInput").ap()
    wqt_d = nc.dram_tensor("wqt", [128, 4, C], BF16, kind="ExternalInput").ap()
    wv_d = nc.dram_tensor("wv", [128, 4, C], BF16, kind="ExternalInput").ap()
    wot_d = nc.dram_tensor("wot", [128, 4, C], BF16, kind="ExternalInput").ap()
    id_d = nc.dram_tensor("ident", [128, HD], BF16, kind="ExternalInput").ap()
    out_d = nc.dram_tensor(
        "out", [B_PER_CORE, 128, 2, 4, 512], BF16, kind="ExternalOutput"
    ).ap()

    with tile.TileContext(nc) as tc, ExitStack() as ctx:
        wp = ctx.enter_context(tc.tile_pool(name="w", bufs=1))
        xtp = ctx.enter_context(tc.tile_pool(name="xt", bufs=2))
        xcp = ctx.enter_context(tc.tile_pool(name="xc", bufs=2))
        gp = ctx.enter_context(tc.tile_pool(name="g", bufs=2))
        tp = ctx.enter_context(tc.tile_pool(name="t", bufs=2))
        smp = ctx.enter_context(tc.tile_pool(name="sm", bufs=2))
        ap_ = ctx.enter_context(tc.tile_pool(name="apt", bufs=2))
        yp = ctx.enter_context(tc.tile_pool(name="y", bufs=2))
        ogp = ctx.enter_context(tc.tile_pool(name="og", bufs=3))

        # PSUM: 8 banks.  acc (4) rotates through G/T/outT/final chunks
        # (outT needs the depth: each chunk's 4 strided y-copies must hide
        # under 3+ chunks of matmul); s (2) per-head-pair score tiles;
        # tr (1) transposes; a (1) A'T chunks.
        ps_acc = ctx.enter_context(tc.tile_pool(name="psacc", bufs=4, space="PSUM"))
        ps_s = ctx.enter_context(tc.tile_pool(name="pss", bufs=2, space="PSUM"))
        ps_tr = ctx.enter_context(tc.tile_pool(name="pstr", bufs=1, space="PSUM"))
        ps_a = ctx.enter_context(tc.tile_pool(name="psa", bufs=1, space="PSUM"))

        st = [{} for _ in range(B_PER_CORE)]

        def s_gram(b):
            """G[i*128:(i+1)*128, :] = sum_mc xt[mc]-slice^T @ xt[mc].
            Two bank-halves; mc-outer inside each half so the first matmul
            of the kernel gates on only the first DMA'd xt chunks."""
            xt_sb = st[b]["xt"]
            g_sb = gp.tile([128, 4, C], BF16, tag="g", name=f"g{b}")
            st[b]["g"] = g_sb
            for half in range(2):
                pts = [
                    ps_acc.tile([128, C], F32, tag="acc", name=f"pg{b}_{half}_{i}")
                    for i in range(2)
                ]
                for mc in range(8):
                    for i in range(2):
                        ic = 2 * half + i
                        nc.tensor.matmul(
                            pts[i][:, :],
                            xt_sb[:, mc, ic * 128 : (ic + 1) * 128],
                            xt_sb[:, mc, :],
                            start=(mc == 0), stop=(mc == 7),
                        )
                for i in range(2):
                    ic = 2 * half + i
                    if ic % 2 == 0:
                        nc.vector.tensor_copy(g_sb[:, ic, :], pts[i][:, :])
                    else:
                        nc.scalar.copy(g_sb[:, ic, :], pts[i][:, :])

        def s_t(b):
            """T = G @ WkT; lhs g_sb[j][:, i-slice] is G[c' in j, c in i]."""
            g_sb = st[b]["g"]
            wkt_sb = st[b]["wkt"]
            t_sb = tp.tile([128, 4, C], BF16, tag="t", name=f"t{b}")
            st[b]["t"] = t_sb
            for i in range(4):
                pt = ps_acc.tile([128, C], F32, tag="acc", name=f"pt{b}_{i}")
                for j in range(4):
                    nc.tensor.matmul(
                        pt[:, :],
                        g_sb[:, j, i * 128 : (i + 1) * 128],
                        wkt_sb[:, j, :],
                        start=(j == 0), stop=(j == 3),
                    )
                if i % 2 == 0:
                    nc.vector.tensor_copy(t_sb[:, i, :], pt[:, :])
                else:
                    nc.scalar.copy(t_sb[:, i, :], pt[:, :])

        def s_scores(b):
            """scores per head-pair hp in its own PSUM tile; softmax over e."""
            wqt_sb = st[b]["wqt"]
            t_sb = st[b]["t"]
            es = smp.tile([128, 4, HD], BF16, tag="es", name=f"es{b}")
            rs = smp.tile([128, 4, 1], F32, tag="rs", name=f"rs{b}")
            rcp = smp.tile([128, 4, 1], F32, tag="rcp", name=f"rcp{b}")
            at1 = smp.tile([128, 4, HD], BF16, tag="at1", name=f"at1_{b}")
            at_bd = smp.tile([128, 4, 128], BF16, tag="atbd", name=f"at_bd{b}")
            nc.vector.memset(at_bd[:, :, :], 0.0)
            for hp in range(4):
                ps1 = ps_s.tile([128, 128], F32, tag="s1", name=f"ps1_{b}_{hp}")
                for j in range(4):
                    nc.tensor.matmul(
                        ps1[:, :],
                        wqt_sb[:, j, hp * 128 : (hp + 1) * 128],
                        t_sb[:, j, hp * 128 : (hp + 1) * 128],
                        start=(j == 0), stop=(j == 3),
                    )
                for hh in range(2):
                    psl = slice(hh * 64, hh * 64 + 64)
                    nc.scalar.activation(
                        es[psl, hp, :], ps1[psl, psl],
                        AF.Exp, scale=0.125,
                        accum_out=rs[psl, hp, :],
                    )
                nc.vector.reciprocal(rcp[:, hp, :], rs[:, hp, :])
                nc.scalar.activation(
                    at1[:, hp, :], es[:, hp, :], AF.Copy, scale=rcp[:, hp, :],
                )
            st[b]["at1"], st[b]["at_bd"] = at1, at_bd

        def s_transpose(b):
            at1, at_bd = st[b]["at1"], st[b]["at_bd"]
            pst = ps_tr.tile([64, 8, HD], BF16, tag="tr", name=f"pst{b}")
            for hp in range(4):
                for hh in range(2):
                    h = 2 * hp + hh
                    psl = slice(hh * 64, hh * 64 + 64)
                    nc.tensor.transpose(
                        pst[:, h, :], at1[psl, hp, :], ident[psl, :]
                    )
                    if hh == 0:
                        nc.vector.tensor_copy(at_bd[psl, hp, psl], pst[:, h, :])
                    else:
                        nc.scalar.copy(at_bd[psl, hp, psl], pst[:, h, :])

        def s_apt(b):
            """A'T[i-chunk, (h,d)] = wv-chunks^T @ blockdiag(attn^T)."""
            wv_sb = st[b]["wv"]
            at_bd = st[b]["at_bd"]
            apt_sb = ap_.tile([128, 4, C], BF16, tag="apt", name=f"apt{b}")
            st[b]["apt"] = apt_sb
            for i in range(4):
                pa = ps_a.tile([128, C], F32, tag="a", name=f"pa{b}_{i}")
                for hp in range(4):
                    nc.tensor.matmul(
                        pa[:, hp * 128 : (hp + 1) * 128],
                        wv_sb[:, hp, i * 128 : (i + 1) * 128],
                        at_bd[:, hp, :],
                        start=True, stop=True,
                    )
                if i % 2 == 0:
                    nc.vector.tensor_copy(apt_sb[:, i, :], pa[:, :])
                else:
                    nc.scalar.copy(apt_sb[:, i, :], pa[:, :])

        def s_outt(b):
            """outT[mc-chunk, (h,d)] = sum_j xc[j, mc-slice]^T @ A'T[j].
            The psum->y copies realize the transpose(2,3) scramble:
            y[(h%2)*64+a, h//2, (2mc+rr)*64 + d] = outT[rr*64+a, (h,d)]."""
            xc_sb = st[b]["xc"]
            apt_sb = st[b]["apt"]
            y_sb = yp.tile([128, 4, N], BF16, tag="y", name=f"y{b}")
            st[b]["y"] = y_sb
            for mc in range(8):
                po = ps_acc.tile([128, 4, 128], F32, tag="acc", name=f"po{b}_{mc}")
                for j in range(4):
                    nc.tensor.matmul(
                        po[:, :, :],
                        xc_sb[:, j, mc * 128 : (mc + 1) * 128],
                        apt_sb[:, j, :],
                        start=(j == 0), stop=(j == 3),
                    )
                k = 0
                for rr in range(2):
                    for par in range(2):
                        src = po[
                            rr * 64 : rr * 64 + 64, :, par * 64 : par * 64 + 64
                        ]
                        dst = y_sb[
                            par * 64 : par * 64 + 64,
                            :,
                            (2 * mc + rr) * 64 : (2 * mc + rr) * 64 + 64,
                        ]
                        if k % 2 == 0:
                            nc.vector.tensor_copy(dst, src)
                        else:
                            nc.scalar.copy(dst, src)
                        k += 1

        def s_final(b):
            """final[oc-chunk, sh-half] = sum_j wot[j, oc-slice]^T @ y[j, sh].
            sh-outer: the sh=0 half gates on outT chunks 0-3 only."""
            wot_sb = st[b]["wot"]
            y_sb = st[b]["y"]
            for sh in range(2):
                og = ogp.tile([128, 4, 512], BF16, tag="og", name=f"og{b}_{sh}")
                for oc in range(4):
                    pf = ps_acc.tile(
                        [128, C], F32, tag="acc", name=f"pf{b}_{sh}_{oc}"
                    )
                    for j in range(4):
                        nc.tensor.matmul(
                            pf[:, :],
                            wot_sb[:, j, oc * 128 : (oc + 1) * 128],
                            y_sb[:, j, sh * 512 : (sh + 1) * 512],
                            start=(j == 0), stop=(j == 3),
                        )
                    if oc % 2 == 0:
                        nc.vector.tensor_copy(og[:, oc, :], pf[:, :])
                    else:
                        nc.scalar.copy(og[:, oc, :], pf[:, :])
                    eng = nc.sync if oc % 2 == 0 else nc.scalar
                    eng.dma_start(
                        out_d[b, :, sh, oc : oc + 1, :], og[:, oc : oc + 1, :]
                    )

        # ---- loads ----
        # both HWDGE queues split batch-0 x with >=2KB rows; gpsimd SWDGE
        # carries batch-1 x + ident (8KB rows, ~300GB/s).
        ident = wp.tile([128, HD], BF16, tag="ident", name="ident_sb")
        xt0 = xtp.tile([128, 8, C], BF16, tag="xt", name="xt_sb0")
        xt1 = xtp.tile([128, 8, C], BF16, tag="xt", name="xt_sb1")
        xc0 = xcp.tile([128, 4, N], BF16, tag="xc", name="xc_sb0")
        xc1 = xcp.tile([128, 4, N], BF16, tag="xc", name="xc_sb1")
        st[0]["xt"], st[1]["xt"] = xt0, xt1
        st[0]["xc"], st[1]["xc"] = xc0, xc1

        nc.sync.dma_start(xt0[:, 0:2, :], xt_d[0, :, 0:2, :])
        nc.sync.dma_start(xt0[:, 2:4, :], xt_d[0, :, 2:4, :])
        nc.scalar.dma_start(xt0[:, 4:8, :], xt_d[0, :, 4:8, :])
        nc.sync.dma_start(xc0[:, :, :], xc_d[0, :, :, :])

        w_sb = {}
        for wname, d in (("wkt", wkt_d), ("wqt", wqt_d), ("wv", wv_d), ("wot", wot_d)):
            t = wp.tile([128, 4, C], BF16, tag=wname, name=f"w_{wname}")
            w_sb[wname] = (t, d)
            for b in range(B_PER_CORE):
                st[b][wname] = t
        nc.scalar.dma_start(w_sb["wkt"][0][:, :, :], wkt_d[:, :, :])
        nc.scalar.dma_start(w_sb["wqt"][0][:, :, :], wqt_d[:, :, :])
        nc.gpsimd.dma_start(ident[:, :], id_d)

        # loads not needed before ~20us are token-gated on gpsimd's queue
        # so their packets don't steal DMA-engine slots from the xt0/wkt
        # loads that feed the first three phases (the 16 HW DMA engines
        # round-robin all queues' packets).
        def _gated_load(dst_tile, dst_full, src, gate_ap):
            nc.gpsimd.tensor_copy(dst_tile[0:1, 0, 0:1], gate_ap)
            nc.gpsimd.dma_start(dst_full, src)

        # ---- schedule ----
        s_gram(0)
        g0 = st[0]["g"]
        _gated_load(xt1, xt1[:, :, :], xt_d[1, :, :, :], g0[0:1, 0, 0:1])
        _gated_load(
            w_sb["wv"][0], w_sb["wv"][0][:, :, :], wv_d[:, :, :], g0[0:1, 2, 0:1]
        )
        s_t(0)
        s_scores(0)
        at1_0 = st[0]["at1"]
        _gated_load(
            w_sb["wot"][0], w_sb["wot"][0][:, :, :], wot_d[:, :, :],
            at1_0[0:1, 0, 0:1],
        )
        s_gram(1)          # softmax(0) hides under G1+T1
        s_t(1)
        s_transpose(0)
        s_apt(0)
        apt0 = st[0]["apt"]
        _gated_load(xc1, xc1[:, :, :], xc_d[1, :, :, :], apt0[0:1, 0, 0:1])
        s_outt(0)
        s_scores(1)
        s_transpose(1)     # at_bd/apt copies queue ahead of og0 copies
        s_apt(1)
        s_final(0)         # softmax(1) hid under O0
        s_outt(1)
        s_final(1)

    nc.compile()
    _split_excess_dma_waits(nc)
    return nc


_PROGRAM = None


def _get_program():
    global _PROGRAM
    if _PROGRAM is None:
        _PROGRAM = build_program()
    return _PROGRAM


def make_in_maps(x, Wq, Wk, Wv, Wo):
    import ml_dtypes

    bf = ml_dtypes.bfloat16
    # permute spatial axis n = 16a + r -> m = 64r + a
    xm = (
        x.reshape(16, C, 64, 16)
        .transpose(0, 1, 3, 2)
        .reshape(16, C, N)
    )
    # xc: [b, 128, cc, m]  (X, channel-partition)
    xc = np.ascontiguousarray(
        xm.reshape(16, 4, 128, N).transpose(0, 2, 1, 3).astype(bf)
    )
    # xt: [b, 128, mc, c]  (X^T, m-partition)
    xt = np.ascontiguousarray(
        xm.transpose(0, 2, 1).reshape(16, 8, 128, C).transpose(0, 2, 1, 3).astype(bf)
    )

    def _wt(W):
        return np.ascontiguousarray(
            W.T.reshape(4, 128, C).transpose(1, 0, 2).astype(bf)
        )

    wkt, wqt, wot = _wt(Wk), _wt(Wq), _wt(Wo)
    # wv: [128 (hh,e), hp, c]: row (2*hp+hh)*64+e of Wv
    wv = np.ascontiguousarray(
        Wv.reshape(4, 2, HD, C).transpose(1, 2, 0, 3).reshape(128, 4, C).astype(bf)
    )
    ident = np.vstack([np.eye(HD), np.eye(HD)]).astype(bf)
    in_maps = []
    for c in range(N_CORES):
        bsl = slice(c * B_PER_CORE, (c + 1) * B_PER_CORE)
        in_maps.append(
            {
                "xt": np.ascontiguousarray(xt[bsl]),
                "xc": np.ascontiguousarray(xc[bsl]),
                "wkt": wkt,
                "wqt": wqt,
                "wv": wv,
                "wot": wot,
                "ident": ident,
            }
        )
    return in_maps


def kernel(x, Wq, bq, Wk, bk, Wv, bv, Wo, bo, _trace=False):
    # biases are zeros by construction in this problem (spec fill="zeros");
    # they are not applied on-device.
    nc = _get_program()
    in_maps = make_in_maps(x, Wq, Wk, Wv, Wo)
    res = run_bass_kernel_spmd(nc, in_maps, list(range(N_CORES)), trace=_trace)
    outs = [
        np.asarray(res.results[c]["out"]).astype(np.float32) for c in range(N_CORES)
    ]
    # out [b, p, sh, oc, s] -> F[b, oc*128+p, sh*512+s]; the free index is
    # already the true spatial-flat index (no inverse permutation needed)
    full = (
        np.concatenate(outs, axis=0)
        .transpose(0, 3, 1, 2, 4)
        .reshape(16, C, N)
        .reshape(16, C, 32, 32)
    )
    if _trace:
        return full, res
    return full


# revision 11
# speedup vs baseline: 1.0227x; 1.0227x over previous
"""Trainium2 Bass kernel for nn_MultiHeadAttentionBlock_49967649521921.

Reference computation (per batch b, x viewed as [C=512, N=1024]):
    q = Wq @ x ; k = Wk @ x ; v = Wv @ x          (1x1 convs, biases zero)
    per head h (8 heads, hd=64):
      scores[d,e] = sum_n q_h[d,n] k_h[e,n] / 8
      attn = softmax(scores, axis=e)
      out_h[d,n]  = sum_e attn[d,e] v_h[e,n]
    y[c',s'] with c' = h*64 + n//16, s' = (n%16)*64 + d
    final = Wo @ y    -> reshape [512, 32, 32]

v4 design — Gram-matrix restructure. Because scores contract over the
SPATIAL axis, q/k projections fold into one shared Gram matrix:
    G  = X X^T                  (per batch, [512,512])
    T  = G Wk^T                 ([512,512])
    scores_hp = Wq^T-slices vs T-slices    (per head-pair [128,128],
                                            one PSUM bank per pair so the
                                            Act-exp reads never stall the
                                            next pair's matmuls)
    attn      = softmax (Act exp + accum, DVE recip, Act copy-scale)
    A'^T[c,(h,d)] = Wv^T-chunks @ blockdiag(attn^T)
    out^T[m,(h,d)] = X^T A'^T   (m-order n=16a+r -> m=64r+a makes the
                                 reference transpose(2,3) a strided copy)
    final = Wo^T-chunks^T @ y   (sh-outer so each half gates on half of y)

Schedule interleaves the two batches so softmax latency always hides
under >6us of independent matmul work:
    G0 T0 S0 | G1 T1 | X0 A0 O0 | S1 | F0 | X1 A1 O1 | F1
Loads: both HWDGE queues carry batch-0 x in >=2KB/row slices (the 1KB
descriptor path runs at ~75GB/s and starves G0); gpsimd SWDGE carries
batch-1 x.
"""

import os
import sys

import numpy as np

for _p in ("/opt/trn_rl_repo",):
    if _p not in sys.path and os.path.isdir(_p):
        sys.path.insert(0, _p)

from contextlib import ExitStack

import concourse.bass as bass
import concourse.tile as tile
from concourse import bacc
from concourse import mybir
from concourse.bass_utils import run_bass_kernel_spmd

F32 = mybir.dt.float32
BF16 = mybir.dt.bfloat16
AF = mybir.ActivationFunctionType

N_CORES = 8
B_PER_CORE = 2
C = 512
N = 1024
NH = 8
HD = 64


def _split_excess_dma_waits(nc):
    """walrus' static-DMA (PSEUDO_DMA_DIRECT2D) encoding accepts a single
    sync-wait; Bacc's generate_event_semaphores only splits waits on compute
    instructions. Move excess DMA waits onto preceding EventSemaphore
    carriers (2 waits each) on the same engine queue."""
    for f in nc.m.functions:
        for blk in f.blocks:
            changed = False
            new_insts = []
            for inst in blk.instructions:
                si = inst.sync_info
                waits = list(si.on_wait) if si is not None and si.on_wait else []
                if inst.opcode == "DMACopy" and len(waits) > 1:
                    keep, excess = waits[:1], waits[1:]
                    k = 0
                    while excess:
                        chunk, excess = excess[:2], excess[2:]
                        ev = mybir.InstEventSemaphore(
                            name=f"{inst.name}-evw{k}",
                            opcode="EventSemaphore",
                            engine=inst.engine,
                            sync_info=mybir.SyncInfo(on_wait=chunk, on_update=[]),
                        )
                        new_insts.append(ev)
                        k += 1
                    inst.sync_info = mybir.SyncInfo(
                        on_wait=keep, on_update=list(si.on_update or [])
                    )
                    changed = True
                new_insts.append(inst)
            if changed:
                blk.instructions = new_insts


def build_program():
    nc = bacc.Bacc("TRN2", target_bir_lowering=False, debug=False)

    xt_d = nc.dram_tensor("xt", [B_PER_CORE, 128, 8, C], BF16, kind="ExternalInput").ap()
    xc_d = nc.dram_tensor("xc", [B_PER_CORE, 128, 4, N], BF16, kind="ExternalInput").ap()
    wkt_d = nc.dram_tensor("wkt", [128, 4, C], BF16, kind="ExternalInput").ap()
    wqt_d = nc.dram_tensor("wqt", [128, 4, C], BF16, kind="ExternalInput").ap()
    wv_d = nc.dram_tensor("wv", [128, 4, C], BF16, kind="ExternalInput").ap()
    wot_d = nc.dram_tensor("wot", [128, 4, C], BF16, kind="ExternalInput").ap()
    id_d = nc.dram_tensor("ident", [128, HD], BF16, kind="ExternalInput").ap()
    out_d = nc.dram_tensor(
        "out", [B_PER_CORE, 128, 2, 4, 512], BF16, kind="ExternalOutput"
    ).ap()

    with tile.TileContext(nc) as tc, ExitStack() as ctx:
        wp = ctx.enter_context(tc.tile_pool(name="w", bufs=1))
        xtp = ctx.enter_context(tc.tile_pool(name="xt", bufs=2))
        xcp = ctx.enter_context(tc.tile_pool(name="xc", bufs=2))
        gp = ctx.enter_context(tc.tile_pool(name="g", bufs=2))
        tp = ctx.enter_context(tc.tile_pool(name="t", bufs=2))
        smp = ctx.enter_context(tc.tile_pool(name="sm", bufs=2))
        ap_ = ctx.enter_context(tc.tile_pool(name="apt", bufs=2))
        yp = ctx.enter_context(tc.tile_pool(name="y", bufs=2))
        ogp = ctx.enter_context(tc.tile_pool(name="og", bufs=3))

        # PSUM: 8 banks.  acc (4) rotates through G/T/outT/final chunks
        # (outT needs the depth: each chunk's 4 strided y-copies must hide
        # under 3+ chunks of matmul); s (2) per-head-pair score tiles;
        # tr (1) transposes; a (1) A'T chunks.
        ps_acc = ctx.enter_context(tc.tile_pool(name="psacc", bufs=4, space="PSUM"))
        ps_s = ctx.enter_context(tc.tile_pool(name="pss", bufs=2, space="PSUM"))
        ps_tr = ctx.enter_context(tc.tile_pool(name="pstr", bufs=1, space="PSUM"))
        ps_a = ctx.enter_context(tc.tile_pool(name="psa", bufs=1, space="PSUM"))

        st = [{} for _ in range(B_PER_CORE)]

        def s_gram(b):
            """G[i*128:(i+1)*128, :] = sum_mc xt[mc]-slice^T @ xt[mc]."""
            xt_sb = st[b]["xt"]
            g_sb = gp.tile([128, 4, C], BF16, tag="g", name=f"g{b}")
            st[b]["g"] = g_sb
            pts = [
                ps_acc.tile([128, C], F32, tag="acc", name=f"pg{b}_{i}")
                for i in range(4)
            ]
            for mc in range(8):
                for i in range(4):
                    nc.tensor.matmul(
                        pts[i][:, :],
                        xt_sb[:, mc, i * 128 : (i + 1) * 128],
                        xt_sb[:, mc, :],
                        start=(mc == 0), stop=(mc == 7),
                    )
            for i in range(4):
                if i % 2 == 0:
                    nc.vector.tensor_copy(g_sb[:, i, :], pts[i][:, :])
                else:
                    nc.scalar.copy(g_sb[:, i, :], pts[i][:, :])

        def s_t(b):
            """T = G @ WkT; lhs g_sb[j][:, i-slice] is G[c' in j, c in i]."""
            g_sb = st[b]["g"]
            wkt_sb = st[b]["wkt"]
            t_sb = tp.tile([128, 4, C], BF16, tag="t", name=f"t{b}")
            st[b]["t"] = t_sb
            for i in range(4):
                pt = ps_acc.tile([128, C], F32, tag="acc", name=f"pt{b}_{i}")
                for j in range(4):
                    nc.tensor.matmul(
                        pt[:, :],
                        g_sb[:, j, i * 128 : (i + 1) * 128],
                        wkt_sb[:, j, :],
                        start=(j == 0), stop=(j == 3),
                    )
                if i % 2 == 0:
                    nc.vector.tensor_copy(t_sb[:, i, :], pt[:, :])
                else:
                    nc.scalar.copy(t_sb[:, i, :], pt[:, :])

        def s_scores(b):
            """scores per head-pair hp in its own PSUM tile; softmax over e."""
            wqt_sb = st[b]["wqt"]
            t_sb = st[b]["t"]
            es = smp.tile([128, 4, HD], BF16, tag="es", name=f"es{b}")
            rs = smp.tile([128, 4, 1], F32, tag="rs", name=f"rs{b}")
            rcp = smp.tile([128, 4, 1], F32, tag="rcp", name=f"rcp{b}")
            at1 = smp.tile([128, 4, HD], BF16, tag="at1", name=f"at1_{b}")
            at_bd = smp.tile([128, 4, 128], BF16, tag="atbd", name=f"at_bd{b}")
            nc.vector.memset(at_bd[:, :, :], 0.0)
            for hp in range(4):
                ps1 = ps_s.tile([128, 128], F32, tag="s1", name=f"ps1_{b}_{hp}")
                for j in range(4):
                    nc.tensor.matmul(
                        ps1[:, :],
                        wqt_sb[:, j, hp * 128 : (hp + 1) * 128],
                        t_sb[:, j, hp * 128 : (hp + 1) * 128],
                        start=(j == 0), stop=(j == 3),
                    )
                for hh in range(2):
                    psl = slice(hh * 64, hh * 64 + 64)
                    nc.scalar.activation(
                        es[psl, hp, :], ps1[psl, psl],
                        AF.Exp, scale=0.125,
                        accum_out=rs[psl, hp, :],
                    )
                nc.vector.reciprocal(rcp[:, hp, :], rs[:, hp, :])
                nc.scalar.activation(
                    at1[:, hp, :], es[:, hp, :], AF.Copy, scale=rcp[:, hp, :],
                )
            st[b]["at1"], st[b]["at_bd"] = at1, at_bd

        def s_transpose(b):
            at1, at_bd = st[b]["at1"], st[b]["at_bd"]
            pst = ps_tr.tile([64, 8, HD], BF16, tag="tr", name=f"pst{b}")
            for hp in range(4):
                for hh in range(2):
                    h = 2 * hp + hh
                    psl = slice(hh * 64, hh * 64 + 64)
                    nc.tensor.transpose(
                        pst[:, h, :], at1[psl, hp, :], ident[psl, :]
                    )
                    if hh == 0:
                        nc.vector.tensor_copy(at_bd[psl, hp, psl], pst[:, h, :])
                    else:
                        nc.scalar.copy(at_bd[psl, hp, psl], pst[:, h, :])

        def s_apt(b):
            """A'T[i-chunk, (h,d)] = wv-chunks^T @ blockdiag(attn^T)."""
            wv_sb = st[b]["wv"]
            at_bd = st[b]["at_bd"]
            apt_sb = ap_.tile([128, 4, C], BF16, tag="apt", name=f"apt{b}")
            st[b]["apt"] = apt_sb
            for i in range(4):
                pa = ps_a.tile([128, C], F32, tag="a", name=f"pa{b}_{i}")
                for hp in range(4):
                    nc.tensor.matmul(
                        pa[:, hp * 128 : (hp + 1) * 128],
                        wv_sb[:, hp, i * 128 : (i + 1) * 128],
                        at_bd[:, hp, :],
                        start=True, stop=True,
                    )
                if i % 2 == 0:
                    nc.vector.tensor_copy(apt_sb[:, i, :], pa[:, :])
                else:
                    nc.scalar.copy(apt_sb[:, i, :], pa[:, :])

        def s_outt(b):
            """outT[mc-chunk, (h,d)] = sum_j xc[j, mc-slice]^T @ A'T[j].
            The psum->y copies realize the transpose(2,3) scramble:
            y[(h%2)*64+a, h//2, (2mc+rr)*64 + d] = outT[rr*64+a, (h,d)]."""
            xc_sb = st[b]["xc"]
            apt_sb = st[b]["apt"]
            y_sb = yp.tile([128, 4, N], BF16, tag="y", name=f"y{b}")
            st[b]["y"] = y_sb
            for mc in range(8):
                po = ps_acc.tile([128, 4, 128], F32, tag="acc", name=f"po{b}_{mc}")
                for j in range(4):
                    nc.tensor.matmul(
                        po[:, :, :],
                        xc_sb[:, j, mc * 128 : (mc + 1) * 128],
                        apt_sb[:, j, :],
                        start=(j == 0), stop=(j == 3),
                    )
                k = 0
                for rr in range(2):
                    for par in range(2):
                        src = po[
                            rr * 64 : rr * 64 + 64, :, par * 64 : par * 64 + 64
                        ]
                        dst = y_sb[
                            par * 64 : par * 64 + 64,
                            :,
                            (2 * mc + rr) * 64 : (2 * mc + rr) * 64 + 64,
                        ]
                        if k % 2 == 0:
                            nc.vector.tensor_copy(dst, src)
                        else:
                            nc.scalar.copy(dst, src)
                        k += 1

        def s_final(b):
            """final[oc-chunk, :] = sum_j wot[j, oc-slice]^T @ y[j, :].
            j-outer / sh-inner shares each LDWEIGHTS across two matmuls.
            Output DMA per sh-half tile; the last batch's are split by
            partition range across queues (any [128,..] DMA costs ~4us of
            per-queue descriptor dispatch; halves run in parallel)."""
            wot_sb = st[b]["wot"]
            y_sb = st[b]["y"]
            ogs = [
                ogp.tile([128, 4, 512], BF16, tag="og", name=f"og{b}_{sh}")
                for sh in range(2)
            ]
            for oc in range(4):
                pf = [
                    ps_acc.tile([128, C], F32, tag="acc", name=f"pf{b}_{oc}_{sh}")
                    for sh in range(2)
                ]
                for j in range(4):
                    for sh in range(2):
                        nc.tensor.matmul(
                            pf[sh][:, :],
                            wot_sb[:, j, oc * 128 : (oc + 1) * 128],
                            y_sb[:, j, sh * 512 : (sh + 1) * 512],
                            start=(j == 0), stop=(j == 3),
                        )
                for sh in range(2):
                    if (oc + sh) % 2 == 0:
                        nc.vector.tensor_copy(ogs[sh][:, oc, :], pf[sh][:, :])
                    else:
                        nc.scalar.copy(ogs[sh][:, oc, :], pf[sh][:, :])
            if b == 0:
                nc.sync.dma_start(out_d[0, :, 0, :, :], ogs[0][:, :, :])
                nc.scalar.dma_start(out_d[0, :, 1, :, :], ogs[1][:, :, :])
            else:
                nc.sync.dma_start(out_d[1, 0:64, 0, :, :], ogs[0][0:64, :, :])
                nc.scalar.dma_start(out_d[1, 64:128, 0, :, :], ogs[0][64:128, :, :])
                nc.gpsimd.dma_start(out_d[1, :, 1, :, :], ogs[1][:, :, :])

        # ---- loads ----
        # both HWDGE queues split batch-0 x with >=2KB rows; gpsimd SWDGE
        # carries batch-1 x + ident (8KB rows, ~300GB/s).
        ident = wp.tile([128, HD], BF16, tag="ident", name="ident_sb")
        xt0 = xtp.tile([128, 8, C], BF16, tag="xt", name="xt_sb0")
        xt1 = xtp.tile([128, 8, C], BF16, tag="xt", name="xt_sb1")
        xc0 = xcp.tile([128, 4, N], BF16, tag="xc", name="xc_sb0")
        xc1 = xcp.tile([128, 4, N], BF16, tag="xc", name="xc_sb1")
        st[0]["xt"], st[1]["xt"] = xt0, xt1
        st[0]["xc"], st[1]["xc"] = xc0, xc1

        # xt0 gates the first matmul: split it by PARTITION range across
        # all three queues (descriptor dispatch is ~30ns/row per queue, so
        # a [128,..] load costs ~4us on one queue but ~1.4us on three).
        # Everything else is one full-tile load, FIFO-staggered in
        # first-use order; queues dispatch mostly independently.
        nc.sync.dma_start(xt0[0:48, :, :], xt_d[0, 0:48, :, :])
        nc.scalar.dma_start(xt0[48:96, :, :], xt_d[0, 48:96, :, :])
        nc.gpsimd.dma_start(xt0[96:128, :, :], xt_d[0, 96:128, :, :])

        w_sb = {}
        for wname in ("wkt", "wqt", "wv", "wot"):
            t = wp.tile([128, 4, C], BF16, tag=wname, name=f"w_{wname}")
            w_sb[wname] = t
            for b in range(B_PER_CORE):
                st[b][wname] = t
        nc.sync.dma_start(xc0[:, :, :], xc_d[0, :, :, :])
        nc.scalar.dma_start(w_sb["wkt"][:, :, :], wkt_d[:, :, :])
        nc.scalar.dma_start(w_sb["wqt"][:, :, :], wqt_d[:, :, :])
        nc.gpsimd.dma_start(xt1[:, :, :], xt_d[1, :, :, :])
        nc.gpsimd.dma_start(w_sb["wv"][:, :, :], wv_d[:, :, :])
        nc.gpsimd.dma_start(ident[:, :], id_d)
        nc.gpsimd.dma_start(w_sb["wot"][:, :, :], wot_d[:, :, :])
        nc.gpsimd.dma_start(xc1[:, :, :], xc_d[1, :, :, :])

        # ---- schedule ----
        s_gram(0)
        s_t(0)
        s_scores(0)
        s_gram(1)          # softmax(0) hides under G1+T1
        s_t(1)
        s_transpose(0)
        s_apt(0)
        s_outt(0)
        s_scores(1)
        s_transpose(1)     # at_bd/apt copies queue ahead of og0 copies
        s_apt(1)
        s_final(0)         # softmax(1) hid under O0
        s_outt(1)
        s_final(1)

    nc.compile()
    _split_excess_dma_waits(nc)
    return nc


_PROGRAM = None


def _get_program():
    global _PROGRAM
    if _PROGRAM is None:
        _PROGRAM = build_program()
    return _PROGRAM


def make_in_maps(x, Wq, Wk, Wv, Wo):
    import ml_dtypes

    bf = ml_dtypes.bfloat16
    # permute spatial axis n = 16a + r -> m = 64r + a
    xm = (
        x.reshape(16, C, 64, 16)
        .transpose(0, 1, 3, 2)
        .reshape(16, C, N)
    )
    # xc: [b, 128, cc, m]  (X, channel-partition)
    xc = np.ascontiguousarray(
        xm.reshape(16, 4, 128, N).transpose(0, 2, 1, 3).astype(bf)
    )
    # xt: [b, 128, mc, c]  (X^T, m-partition)
    xt = np.ascontiguousarray(
        xm.transpose(0, 2, 1).reshape(16, 8, 128, C).transpose(0, 2, 1, 3).astype(bf)
    )

    def _wt(W):
        return np.ascontiguousarray(
            W.T.reshape(4, 128, C).transpose(1, 0, 2).astype(bf)
        )

    wkt, wqt, wot = _wt(Wk), _wt(Wq), _wt(Wo)
    # wv: [128 (hh,e), hp, c]: row (2*hp+hh)*64+e of Wv
    wv = np.ascontiguousarray(
        Wv.reshape(4, 2, HD, C).transpose(1, 2, 0, 3).reshape(128, 4, C).astype(bf)
    )
    ident = np.vstack([np.eye(HD), np.eye(HD)]).astype(bf)
    in_maps = []
    for c in range(N_CORES):
        bsl = slice(c * B_PER_CORE, (c + 1) * B_PER_CORE)
        in_maps.append(
            {
                "xt": np.ascontiguousarray(xt[bsl]),
                "xc": np.ascontiguousarray(xc[bsl]),
                "wkt": wkt,
                "wqt": wqt,
                "wv": wv,
                "wot": wot,
                "ident": ident,
            }
        )
    return in_maps


def kernel(x, Wq, bq, Wk, bk, Wv, bv, Wo, bo, _trace=False):
    # biases are zeros by construction in this problem (spec fill="zeros");
    # they are not applied on-device.
    nc = _get_program()
    in_maps = make_in_maps(x, Wq, Wk, Wv, Wo)
    res = run_bass_kernel_spmd(nc, in_maps, list(range(N_CORES)), trace=_trace)
    outs = [
        np.asarray(res.results[c]["out"]).astype(np.float32) for c in range(N_CORES)
    ]
    # out [b, p, sh, oc, s] -> F[b, oc*128+p, sh*512+s]; the free index is
    # already the true spatial-flat index (no inverse permutation needed)
    full = (
        np.concatenate(outs, axis=0)
        .transpose(0, 3, 1, 2, 4)
        .reshape(16, C, N)
        .reshape(16, C, 32, 32)
    )
    if _trace:
        return full, res
    return full
